# revision 20
# baseline (speedup 1.0000x reference)
"""ContextAttention Trainium2 kernel (8-core data parallel, fp8 DoubleRow).

Computation (per batch row b, S=20, D=300, J=512):
  valid = cumprod(labels != 0)                      prefix-valid mask
  fea   = guide[ann[2b]]                            (host gather, pure data movement)
  pre[s,:] = ctx[b,s,:] @ W_sent.T + b_sent + b_emb + valid[b,s]*(fea @ W_emb.T)
  H = tanh(pre);  scores = H @ w_fc                 (b_fc dropped: softmax shift-invariant)
  attn = renorm(softmax(scores) * (labels != 0))
  out[b,:] = sum_s attn[s] * embedded[b,s,:]

Device layout: batch-major 128-row tiles.  Matmuls run in fp8e4 DoubleRow
(two 128-deep contraction subtiles per instruction at 0.5 cycles/row);
host pre-scales weights x64 to dodge fp8 subnormals, tanh un-scales.
Per s the pre-activation needs contraction depth 301(ctx+bias) + 128
(valid-diag guidance): packed into exactly TWO DoubleRow matmuls by
carrying the ctx tail (d>=256 + bias row) in the guidance matmul's
second subtile (vd4 sub1 = ctx tail, fe8 sub1 = W_sent tail).  The
attn-weighted sum contracts two s per instruction the same way.  tanh is
batched over 4 s in 4-bank PSUM tiles; scores = H @ w_fc runs as f16
multiplies (split DVE/GpSimd) + batched DVE reduces.
"""

import sys
from contextlib import ExitStack

import numpy as np

if "/opt/trn_rl_repo" not in sys.path:
    sys.path.append("/opt/trn_rl_repo")

import concourse.bass as bass
import concourse.tile as tile
from concourse import bacc, mybir
from concourse.bass_utils import run_bass_kernel_spmd

B, S, D, J, VG = 8192, 20, 300, 512, 2078
NCORES = 8
BC = B // NCORES          # 1024 batch rows per core
NBT = BC // 128           # 8 batch tiles per core
WSC = 64.0                # host weight pre-scale (fp8 subnormal avoidance)
NG = S // 4               # 4-s groups per tile
DVE_GROUPS = (0, 3)       # score-multiply groups on DVE; rest on gpsimd
NT = 45                   # ctx tail rows: d=256..299 plus the bias-ones row
F32 = mybir.dt.float32
F16 = mybir.dt.float16
F8 = mybir.dt.float8e4
DR = mybir.MatmulPerfMode.DoubleRow

_NC_CACHE = {}


def _build():
    nc = bacc.Bacc("TRN2", target_bir_lowering=False, debug=False)

    ctxm_d = nc.dram_tensor("ctxm", [NBT, 128, 2 * S * 128], F8, kind="ExternalInput").ap()
    ctxt_d = nc.dram_tensor("ctxt", [NBT, NT, S * 128], F8, kind="ExternalInput").ap()
    feam_d = nc.dram_tensor("feam", [NBT, 128, 2 * 8 * 128], F8, kind="ExternalInput").ap()
    feat_d = nc.dram_tensor("feat", [NBT, 16, 2 * 128], F8, kind="ExternalInput").ap()
    emb_d = nc.dram_tensor("emb", [NBT, 128, S * D], F16, kind="ExternalInput").ap()
    lab_d = nc.dram_tensor("lab", [NBT, 128, S], F32, kind="ExternalInput").ap()
    wstm_d = nc.dram_tensor("wstm", [128, 2 * J], F8, kind="ExternalInput").ap()
    wstt_d = nc.dram_tensor("wstt", [128, J], F8, kind="ExternalInput").ap()
    wetm_d = nc.dram_tensor("wetm", [8, 128, 2 * J], F8, kind="ExternalInput").ap()
    wett_d = nc.dram_tensor("wett", [16, 2 * J], F8, kind="ExternalInput").ap()
    wfc4_d = nc.dram_tensor("wfc4", [128, 4 * J], F16, kind="ExternalInput").ap()
    eyeg_d = nc.dram_tensor("eyeg", [128, 128], F8, kind="ExternalInput").ap()
    eye_d = nc.dram_tensor("eye", [128, 128], F8, kind="ExternalInput").ap()
    out_d = nc.dram_tensor("wc", [NBT, 128, D], F32, kind="ExternalOutput").ap()

    mul = mybir.AluOpType.mult
    add = mybir.AluOpType.add

    with tile.TileContext(nc) as tc, ExitStack() as ctx:
        consts = ctx.enter_context(tc.tile_pool(name="consts", bufs=1))
        ctxp = ctx.enter_context(tc.tile_pool(name="ctxp", bufs=4))
        feap = ctx.enter_context(tc.tile_pool(name="feap", bufs=3))
        embp = ctx.enter_context(tc.tile_pool(name="embp", bufs=3))
        hp = ctx.enter_context(tc.tile_pool(name="hp", bufs=5))
        hwp = ctx.enter_context(tc.tile_pool(name="hwp", bufs=5))
        adp = ctx.enter_context(tc.tile_pool(name="adp", bufs=3))
        sm = ctx.enter_context(tc.tile_pool(name="sm", bufs=6))
        outp = ctx.enter_context(tc.tile_pool(name="outp", bufs=3))
        psp = ctx.enter_context(tc.tile_pool(name="psp", bufs=2, space="PSUM"))

        # ---- constants ----
        wstm_sb = consts.tile([128, 2, J], F8, tag="wstm")
        nc.sync.dma_start(out=wstm_sb, in_=wstm_d.rearrange("p (a j) -> p a j", a=2))
        wetm_sb = []
        for k in range(8):
            t = consts.tile([128, 2, J], F8, tag=f"wetm{k}")
            nc.sync.dma_start(out=t, in_=wetm_d[k].rearrange("p (a j) -> p a j", a=2))
            wetm_sb.append(t)
        wett_sb = consts.tile([16, 2, J], F8, tag="wett")
        nc.sync.dma_start(out=wett_sb, in_=wett_d.rearrange("p (a j) -> p a j", a=2))
        wfc4_sb = consts.tile([128, 4 * J], F16, tag="wfc4")
        nc.sync.dma_start(out=wfc4_sb, in_=wfc4_d)
        eyeg_sb = consts.tile([128, 128], F8, tag="eyeg")
        nc.sync.dma_start(out=eyeg_sb, in_=eyeg_d)
        eye_sb = consts.tile([128, 128], F8, tag="eye")
        nc.sync.dma_start(out=eye_sb, in_=eye_d)

        # fe8: sub0 = fea_emb (per tile, x8), sub1 = W_sent tail rows (const)
        # -> the guidance matmul's two subtiles add valid*fe*64 AND the ctx
        # tail (d>=256 incl. bias) in one instruction.
        fe8 = [consts.tile([128, 2, J], F8, name=f"fe8{i}", tag=f"fe8{i}")
               for i in range(4)]
        for i in range(4):
            nc.sync.dma_start(out=fe8[i][:, 1, :], in_=wstt_d)
        # vd4: sub0 = 8*valid-diag (DVE per tile), sub1 = ctx tail (DMA per
        # tile into rows 0..NT-1; rows NT..127 stay zero from the memset)
        vd4 = [consts.tile([128, S, 2, 128], F8, name=f"vd4{i}", tag=f"vd4{i}")
               for i in range(4)]
        for i in range(4):
            nc.vector.memset(vd4[i][:, :, 1, :], 0.0)

        def tail_stage(nz, scores, emb_sb):
            # masked softmax: scores were pre-masked to -1e4 at nz==0, and
            # |scores| <= ~8 so no max-subtraction is needed.  den comes for
            # free from the exp's accumulator; attn scaling runs on ACT.
            e = sm.tile([128, S], F32, tag="e")
            den = sm.tile([128, 1], F32, tag="den")
            nc.scalar.activation(
                e, scores[:], mybir.ActivationFunctionType.Exp, accum_out=den
            )
            rden = sm.tile([128, 1], F32, tag="rden")
            nc.vector.reciprocal(rden, den)
            attn = sm.tile([128, S], F32, tag="attn")
            nc.scalar.mul(attn, e, rden[:, 0:1])
            ad = adp.tile([128, S, 128], F16, tag="ad")
            nc.gpsimd.tensor_tensor(
                out=ad,
                in0=eye_sb[:].unsqueeze(1).broadcast_to([128, S, 128]),
                in1=attn[:].unsqueeze(2).broadcast_to([128, S, 128]),
                op=mul,
            )
            ps_wc = psp.tile([128, 4 * J], F32, tag="ps")
            for s in range(S):
                nc.tensor.matmul(
                    ps_wc[:, 0:D], ad[:, s], emb_sb[:, s * D : (s + 1) * D],
                    start=(s == 0), stop=(s == S - 1),
                )
            return ps_wc

        def head_stage(bt):
            # DMAs + fea matmuls + masks + valid-diag for tile bt; issued a
            # full iteration early so vd/fe8 never gate the PE's pre-matmuls
            vd = vd4[bt % 4]
            fe = fe8[bt % 4]
            ctxm_sb = ctxp.tile([128, 2, S, 128], F8, tag="ctxm")
            nc.sync.dma_start(
                out=ctxm_sb, in_=ctxm_d[bt].rearrange("p (a s b) -> p a s b", a=2, s=S)
            )
            nc.sync.dma_start(
                out=vd[0:NT, :, 1, :],
                in_=ctxt_d[bt].rearrange("p (s b) -> p s b", s=S),
            )
            feam_sb = feap.tile([128, 2, 8, 128], F8, tag="feam")
            nc.sync.dma_start(
                out=feam_sb, in_=feam_d[bt].rearrange("p (a k b) -> p a k b", a=2, k=8)
            )
            feat_sb = feap.tile([16, 2, 128], F8, tag="feat")
            nc.sync.dma_start(
                out=feat_sb, in_=feat_d[bt].rearrange("p (a b) -> p a b", a=2)
            )
            emb_sb = embp.tile([128, S * D], F16, tag="emb")
            nc.sync.dma_start(out=emb_sb, in_=emb_d[bt])
            lab_sb = sm.tile([128, S], F32, tag="lab")
            nc.sync.dma_start(out=lab_sb, in_=lab_d[bt])

            # fe = fea @ W_emb.T; psum carries x64 (host-scaled wet), fe8
            # keeps x8 (fp8 range), the other x8 rides on the valid-diag
            ps_fe = psp.tile([128, 4 * J], F32, tag="ps")
            for k in range(8):
                nc.tensor.matmul(
                    ps_fe[:, 0:J], feam_sb[:, :, k, :], wetm_sb[k][:],
                    start=(k == 0), stop=False, perf_mode=DR,
                )
            nc.tensor.matmul(
                ps_fe[:, 0:J], feat_sb[:], wett_sb[:], start=False, stop=True,
                perf_mode=DR,
            )
            nc.scalar.activation(
                fe[:, 0, :], ps_fe[:, 0:J], mybir.ActivationFunctionType.Copy,
                scale=0.125,
            )

            # masks: nz = labels != 0 ; valid = cumprod(nz) via scan
            nz = sm.tile([128, S], F32, tag="nz")
            nc.vector.tensor_scalar(nz, lab_sb, 0.0, None, mybir.AluOpType.not_equal)
            valid = sm.tile([128, S], F32, tag="valid")
            nc.vector.tensor_tensor_scan(
                valid, nz, nz, initial=1.0, op0=mul, op1=mybir.AluOpType.bypass
            )

            # valid-diag (x8 eye; fp8 sub0 of the DoubleRow pair)
            nc.vector.tensor_tensor(
                out=vd[:, :, 0, :],
                in0=eyeg_sb[:].unsqueeze(1).broadcast_to([128, S, 128]),
                in1=valid[:].unsqueeze(2).broadcast_to([128, S, 128]),
                op=mul,
            )

            return ctxm_sb, emb_sb, nz

        def mid_stage(bt, ctxm_sb):
            vd = vd4[bt % 4]
            fe = fe8[bt % 4]
            scores = sm.tile([128, S], F32, tag="scores")
            for g in range(NG):
                ps4 = psp.tile([128, 4 * J], F32, tag="ps")
                for si in range(4):
                    s = 4 * g + si
                    pss = ps4[:, si * J : (si + 1) * J]
                    nc.tensor.matmul(
                        pss, ctxm_sb[:, :, s, :], wstm_sb[:],
                        start=True, stop=False, perf_mode=DR,
                    )
                    nc.tensor.matmul(
                        pss, vd[:, s], fe[:], start=False, stop=True, perf_mode=DR,
                    )
                ht4 = hp.tile([128, 4 * J], F16, tag="ht4")
                nc.scalar.activation(
                    ht4, ps4, mybir.ActivationFunctionType.Tanh, scale=1.0 / WSC
                )
                eng = nc.vector if g in DVE_GROUPS else nc.gpsimd
                hw4 = hwp.tile([128, 4, J], F16, tag="hw4")
                eng.tensor_tensor(
                    out=hw4, in0=ht4[:].rearrange("p (a j) -> p a j", a=4),
                    in1=wfc4_sb[:].rearrange("p (a j) -> p a j", a=4), op=mul,
                )
                nc.vector.tensor_reduce(
                    scores[:, 4 * g : 4 * g + 4], hw4[:],
                    axis=mybir.AxisListType.X, op=add,
                )

            return scores

        def mask_scores(nz, scores):
            msk = sm.tile([128, S], F32, tag="msk")
            nc.vector.tensor_scalar(
                msk, nz, 1.0e4, -1.0e4, mul, mybir.AluOpType.add
            )
            sc2 = sm.tile([128, S], F32, tag="sc2")
            nc.vector.tensor_tensor(out=sc2, in0=scores, in1=msk, op=add)
            return sc2

        def do_tail(pbt, pnz, pscores, pemb):
            ps_wc = tail_stage(pnz, pscores, pemb)
            ot = outp.tile([128, D], F32, tag="ot")
            nc.scalar.copy(ot, ps_wc[:, 0:D])
            nc.sync.dma_start(out=out_d[pbt], in_=ot)

        heads = {0: head_stage(0)}
        pending = None
        for bt in range(NBT):
            ctxm_sb, emb_sb, nz = heads.pop(bt)
            if bt + 1 < NBT:
                heads[bt + 1] = head_stage(bt + 1)
            if pending is not None:
                do_tail(*pending)
            scores = mask_scores(nz, mid_stage(bt, ctxm_sb))
            pending = (bt, nz, scores, emb_sb)

        do_tail(*pending)

    nc.compile()
    return nc


def _get_nc():
    if "v3" not in _NC_CACHE:
        _NC_CACHE["v3"] = _build()
    return _NC_CACHE["v3"]


def prep_inputs(context, embedded, input_labels, guide_input, sent_to_image_ann,
                W_sent, b_sent, W_emb, b_emb, w_fc, b_fc):
    """Host-side shard + layout prep. Pure data movement plus weight layout."""
    f8 = mybir.dt.np(F8)
    f16 = np.float16
    context = np.asarray(context, np.float32)
    embedded = np.asarray(embedded, np.float32)
    labels = np.asarray(input_labels)
    guide = np.asarray(guide_input, np.float32)
    ann2 = np.asarray(sent_to_image_ann)[::2]
    fea = guide[ann2]  # (B, VG) row gather

    W_sent = np.asarray(W_sent, np.float32)
    W_emb = np.asarray(W_emb, np.float32)
    bias = np.asarray(b_sent, np.float32) + np.asarray(b_emb, np.float32)

    # W_sent.T x64: main d<256 as DoubleRow pairs [128,2,J] (d = sub*128+p);
    # tail rows d=256..299 + bias row into [128, J] (rows NT..127 zero) --
    # they ride as sub1 of the guidance matmul (fe8 sub1)
    wstm = np.ascontiguousarray(
        (W_sent.T[:256] * WSC).reshape(2, 128, J).transpose(1, 0, 2)
    )
    wstt = np.zeros((128, J), np.float32)
    wstt[: NT - 1] = W_sent.T[256:D] * WSC
    wstt[NT - 1] = bias * WSC

    # W_emb.T padded to 2080 rows, x64: main 8 chunks of 256 (vg =
    # kk*256 + sub*128 + p), tail [16,2,J] (vg = 2048 + sub*16 + p)
    wet = np.zeros((2080, J), np.float32)
    wet[:VG] = W_emb.T * WSC
    wetm = np.ascontiguousarray(
        wet[:2048].reshape(8, 2, 128, J).transpose(0, 2, 1, 3)
    )
    wett = np.ascontiguousarray(wet[2048:].reshape(2, 16, J).transpose(1, 0, 2))

    wfc4 = np.tile(np.asarray(w_fc, np.float32)[None, :], (128, 4))
    eye = np.eye(128, dtype=np.float32)

    wstm8 = wstm.astype(f8).reshape(128, 2 * J)
    wstt8 = wstt.astype(f8)
    wetm8 = wetm.astype(f8).reshape(8, 128, 2 * J)
    wett8 = wett.astype(f8).reshape(16, 2 * J)
    wfc416 = wfc4.astype(f16)
    eye8 = eye.astype(f8)
    eyeg8 = (8.0 * eye).astype(f8)

    in_maps = []
    for c in range(NCORES):
        c0 = c * BC
        # ctx contraction-major: main pairs [128,2,S,128]; tail rows
        # 256..299 + ones row as [NT, S, 128]
        ctx_c = context[c0 : c0 + BC].reshape(NBT, 128, S, D).transpose(0, 3, 2, 1)
        ctxm = np.ascontiguousarray(
            ctx_c[:, :256].reshape(NBT, 2, 128, S, 128).transpose(0, 2, 1, 3, 4)
        )
        ctxt = np.empty((NBT, NT, S, 128), np.float32)
        ctxt[:, : NT - 1] = ctx_c[:, 256:D]
        ctxt[:, NT - 1] = 1.0
        # fea: pad VG to 2080, transpose, pack pairs
        fea_c = np.zeros((BC, 2080), np.float32)
        fea_c[:, :VG] = fea[c0 : c0 + BC]
        fea_t = fea_c.reshape(NBT, 128, 2080).transpose(0, 2, 1)  # [NBT, vg, b]
        feam = fea_t[:, :2048].reshape(NBT, 8, 2, 128, 128).transpose(0, 3, 2, 1, 4)
        feat = fea_t[:, 2048:].reshape(NBT, 2, 16, 128).transpose(0, 2, 1, 3)
        in_maps.append({
            "ctxm": ctxm.astype(f8).reshape(NBT, 128, 2 * S * 128),
            "ctxt": ctxt.astype(f8).reshape(NBT, NT, S * 128),
            "feam": np.ascontiguousarray(feam).astype(f8).reshape(NBT, 128, 2 * 8 * 128),
            "feat": np.ascontiguousarray(feat).astype(f8).reshape(NBT, 16, 2 * 128),
            "emb": embedded[c0 : c0 + BC].astype(f16).reshape(NBT, 128, S * D),
            "lab": labels[c0 : c0 + BC].reshape(NBT, 128, S).astype(np.float32),
            "wstm": wstm8, "wstt": wstt8, "wetm": wetm8, "wett": wett8,
            "wfc4": wfc416, "eye": eye8, "eyeg": eyeg8,
        })
    return in_maps


def kernel(**inputs):
    in_maps = prep_inputs(**inputs)
    nc = _get_nc()
    res = run_bass_kernel_spmd(nc, in_maps, list(range(NCORES)))
    return np.concatenate(
        [res.results[i]["wc"].reshape(BC, D) for i in range(NCORES)], axis=0
    )
